# revision 13
# baseline (speedup 1.0000x reference)
"""EMD (Sinkhorn) loss kernel for Trainium2, 8 NeuronCores.

Reference: for each (q, p) pair of a 128x128 grid, run a 100-iteration
entropic Sinkhorn solve on a 32x32 cost matrix; logits[q,p] = sum(flow*sim)
* (12.5/32).

Exp-domain formulation (matches the jax log-domain reference):
    K = exp((sim-1)/eps);  v0 = 1
    repeat: r_i = sum_j K_ij v_j ; u = a/r ; s_j = sum_i K_ij u_i ; v = b/s
    logits = sum_ij u_i K_ij v_j sim_ij * (T/32)

Sharding: data-parallel over q (16 q / core -> 2048 independent 32x32
problems per core). Within a core, pair (q_l, p_idx) lives in SBUF
partition p_idx, slot q_l — so the b marginal is partition-resident
([128,32], read via a broadcast access pattern) and the a marginal is a
single 512-float vector replicated across partitions by a stride-0 DMA.

End-to-end wall time here is dominated by host->device transfer over the
axon tunnel (~38 MB/s), not device compute (the 100 Sinkhorn iterations
execute in a few ms). So the kernel ships the similarity map as 6-bit
quantized values bit-packed 4-per-3-bytes (12.6 MB instead of 67 MB fp32)
and dequantizes on device: sim_q = (q+0.5)/64, K = exp((sim_q-1)/eps).
Measured end-to-end rel err vs the fp32 reference: 8.7e-3 (gate 2e-2).
Host packing is chunked and overlapped with async device_put transfers.

Execution mirrors bass_utils.run_bass_kernel_spmd's axon path
(bass2jax PJRT custom call, shard_map over 8 cores) but caches the
jitted callable so repeat calls skip retrace/recompile.
"""

import numpy as np

EPS = 0.05
N_ITERS = 100
TEMP = 12.5
Q, P, N1, N2 = 128, 128, 32, 32
N_CORES = 8
QL = Q // N_CORES          # 16 queries per core
FREE = QL * N1 * N2        # 16384 sim values per partition
POT = QL * 32              # 512 potential values per partition
NCHUNK = 4                 # host pack/transfer pipeline depth
QCH = QL // NCHUNK         # 4 queries per chunk
VCH = QCH * N1 * N2        # 4096 values per partition per chunk
BCH = VCH // 4 * 3         # 3072 packed bytes per partition per chunk

QLEVELS = 64               # 6-bit uniform quantization of sim in [0,1)
DEQ_SCALE = 1.0 / (QLEVELS * EPS)            # exp arg scale per level
DEQ_BIAS = (0.5 / QLEVELS - 1.0) / EPS       # exp arg bias (incl. +0.5 deq)
SIM_SCALE = 1.0 / QLEVELS                    # Copy-activation dequant
SIM_BIAS = 0.5 / QLEVELS


def _marginals(lengths, n):
    mask = (np.arange(n)[None, :] < np.asarray(lengths)[:, None]).astype(np.float32)
    w = mask + np.float32(1e-5)
    return w / w.sum(-1, keepdims=True, dtype=np.float32)


def _chunk_bytes(T):
    """Packed bytes per partition for each chunk (slots 4j..4j+4)."""
    return [sum(int(T[l]) * 24 for l in range(QCH * j, QCH * (j + 1)))
            for j in range(NCHUNK)]


def build_program(T, n_iters=N_ITERS):
    """T: 16 per-slot row budgets (baked into the DMA layout). Each slot l
    ships only T[l] of its 32 rows; the rest of the 32x32 block stays at the
    memset default (sim ~ 0.008), which is what masked rows decode to anyway."""
    from contextlib import ExitStack
    from concourse import bacc, tile, mybir

    nc = bacc.Bacc("TRN2", target_bir_lowering=False, debug=False,
                   enable_asserts=False, num_devices=N_CORES)
    f32 = mybir.dt.float32
    u8 = mybir.dt.uint8
    cb = _chunk_bytes(T)
    kp_d = [nc.dram_tensor(f"kp{j}", [128, cb[j]], u8, kind="ExternalInput")
            for j in range(NCHUNK)]
    # marg rows 0..127: b per proto; rows 128..143: this core's a[16, 32]
    m_d = nc.dram_tensor("marg", [128 + QL, N2], f32, kind="ExternalInput")
    out_d = nc.dram_tensor("out", [128, QL], f32, kind="ExternalOutput")

    with tile.TileContext(nc) as tc:
        _emd_body(tc, n_iters, T, kp_d, m_d, out_d)
    nc.compile()
    return nc


def _emd_body(tc, n_iters, T, kp_d, m_d, out_d):
    from contextlib import ExitStack
    from concourse import mybir
    import concourse.bass as bass
    nc = tc.nc
    f32 = mybir.dt.float32
    u8 = mybir.dt.uint8
    ADD = mybir.AluOpType.add
    AND = mybir.AluOpType.bitwise_and
    OR = mybir.AluOpType.bitwise_or
    SHR = mybir.AluOpType.logical_shift_right
    SHL = mybir.AluOpType.logical_shift_left
    X = mybir.AxisListType.X
    XY = mybir.AxisListType.XY
    AF = mybir.ActivationFunctionType

    ctx = ExitStack()
    sp = ctx.enter_context(tc.tile_pool(name="sp", bufs=1))

    kp = sp.tile([128, NCHUNK * BCH], u8, name="kp")
    nc.gpsimd.memset(kp[:], 0)
    for l in range(QL):
        nb = int(T[l]) * 24
        j = l // QCH
        off = sum(int(T[k]) * 24 for k in range(QCH * j, l))
        dst = kp[:]
        sap = kp_d[j].ap()
        nc.sync.dma_start(
            bass.AP(dst.tensor, dst.offset + l * 768, [dst.ap[0], [1, nb]]),
            bass.AP(sap.tensor, sap.offset + off, [sap.ap[0], [1, nb]]))
    bt = sp.tile([128, N2], f32, name="bt")         # b, per-proto partition
    a_all = sp.tile([128, POT], f32, name="a_all")  # a replicated across parts
    map_ = m_d.ap()
    nc.sync.dma_start(bt[:], bass.AP(map_.tensor, map_.offset,
                                     [[N2, 128], [1, N2]]))
    nc.sync.dma_start(a_all[:], bass.AP(map_.tensor, map_.offset + 128 * N2,
                                        [[0, 128], [1, POT]]))

    q6 = sp.tile([128, FREE], u8, name="q6")
    t1 = sp.tile([128, FREE // 4], u8, name="t1")
    t2 = sp.tile([128, FREE // 4], u8, name="t2")
    k = sp.tile([128, FREE], f32, name="k")
    tmp = sp.tile([128, FREE], f32, name="tmp")
    v = sp.tile([128, POT], f32, name="v")
    r = sp.tile([128, POT], f32, name="r")
    ri = sp.tile([128, POT], f32, name="ri")
    u = sp.tile([128, POT], f32, name="u")
    s = sp.tile([128, POT], f32, name="s")
    w = sp.tile([128, POT], f32, name="w")
    outsb = sp.tile([128, QL], f32, name="outsb")
    biast = sp.tile([128, 1], f32, name="biast")
    nc.gpsimd.memset(biast[:], float(DEQ_BIAS))

    NG = FREE // 4  # packed groups: 4 values in 3 bytes

    def pksrc(off):
        ap = kp[:]
        return bass.AP(ap.tensor, ap.offset + off, [ap.ap[0], [3, NG]])

    def updst(off):
        ap = q6[:]
        return bass.AP(ap.tensor, ap.offset + off, [ap.ap[0], [4, NG]])

    ve = nc.vector
    ve.tensor_scalar(out=updst(0), in0=pksrc(0), scalar1=63, scalar2=None, op0=AND)
    ve.tensor_scalar(out=t1[:], in0=pksrc(0), scalar1=6, scalar2=None, op0=SHR)
    ve.tensor_scalar(out=t2[:], in0=pksrc(1), scalar1=15, scalar2=2, op0=AND, op1=SHL)
    ve.tensor_tensor(out=updst(1), in0=t1[:], in1=t2[:], op=OR)
    ve.tensor_scalar(out=t1[:], in0=pksrc(1), scalar1=4, scalar2=None, op0=SHR)
    ve.tensor_scalar(out=t2[:], in0=pksrc(2), scalar1=3, scalar2=4, op0=AND, op1=SHL)
    ve.tensor_tensor(out=updst(2), in0=t1[:], in1=t2[:], op=OR)
    ve.tensor_scalar(out=updst(3), in0=pksrc(2), scalar1=2, scalar2=None, op0=SHR)

    # K = exp((q+0.5)/64/eps - 1/eps)
    nc.scalar.activation(out=k[:], in_=q6[:], func=AF.Exp,
                         scale=float(DEQ_SCALE), bias=biast[:])

    def v4(t):   # [128, QL, N1, N2] view
        return t[:].rearrange("p (l i j) -> p l i j", i=N1, j=N2)

    def p3(t):   # potential [128, POT] viewed [128, QL, 32]
        return t[:].rearrange("p (l x) -> p l x", x=32)

    def mid_bcast(t):
        # t: [128, (l, j)] read as [128, l, i(bcast), j]
        ap = t[:]
        return bass.AP(ap.tensor, ap.offset, [ap.ap[0], [N2, QL], [0, N1], [1, N2]])

    def trail_bcast(t):
        # t: [128, (l, i)] read as [128, (l, i), j(bcast)]
        return t[:].broadcast_to([128, POT, N2])

    def v3(t):   # [128, (l, i), j] view of a big tile
        return t[:].rearrange("p (li j) -> p li j", j=N2)

    def strided_ij(t):
        # big tile [128, (l, i, j)] read as [128, l, j, i] (i innermost)
        ap = t[:]
        return bass.AP(ap.tensor, ap.offset,
                       [ap.ap[0], [N1 * N2, QL], [1, N2], [N2, N1]])

    def bt_bcast():
        # bt: [128, j] read as [128, l(bcast), j]
        ap = bt[:]
        return bass.AP(ap.tensor, ap.offset, [ap.ap[0], [0, QL], [1, N2]])

    for t in range(n_iters):
        if t == 0:
            ve.tensor_reduce(out=p3(r), in_=v4(k), axis=X, op=ADD)
        else:
            ve.tensor_mul(out=p3(v), in0=bt_bcast(), in1=p3(w))
            ve.tensor_mul(out=v4(tmp), in0=v4(k), in1=mid_bcast(v))
            ve.tensor_reduce(out=p3(r), in_=v4(tmp), axis=X, op=ADD)
        ve.reciprocal(out=ri[:], in_=r[:])
        ve.tensor_mul(out=u[:], in0=a_all[:], in1=ri[:])
        ve.tensor_mul(out=v3(tmp), in0=v3(k), in1=trail_bcast(u))
        ve.tensor_reduce(out=p3(s), in_=strided_ij(tmp), axis=X, op=ADD)
        ve.reciprocal(out=w[:], in_=s[:])

    # logits = sum(plan * sim); sim = (q+0.5)/64 rebuilt into k's tile
    ve.tensor_mul(out=p3(v), in0=bt_bcast(), in1=p3(w))
    ve.tensor_mul(out=v4(tmp), in0=v4(k), in1=mid_bcast(v))
    ve.tensor_mul(out=v3(tmp), in0=v3(tmp), in1=trail_bcast(u))
    nc.scalar.activation(out=k[:], in_=q6[:], func=AF.Copy,
                         scale=float(SIM_SCALE), bias=float(SIM_BIAS))
    ve.tensor_mul(out=tmp[:], in0=tmp[:], in1=k[:])
    ve.tensor_reduce(out=outsb[:], in_=v4(tmp), axis=XY, op=ADD)
    ve.tensor_scalar_mul(out=outsb[:], in0=outsb[:], scalar1=float(TEMP / N1))
    nc.sync.dma_start(out_d.ap(), outsb[:])
    ctx.close()


_CACHE = {}


def _get_exec(T):
    """Build the Bass program for row budgets T and wrap it in a cached
    jitted runner. Keyed on T: a new length pattern recompiles once; repeat
    calls with the same pattern (the common case) dispatch with no retrace.

    Mirrors bass_utils.run_bass_kernel_spmd's axon execution path
    (bass2jax _bass_exec_p custom call under shard_map on 8 cores).
    """
    if T in _CACHE:
        return _CACHE[T]

    import jax
    try:
        jax.config.update("jax_compilation_cache_dir", "/tmp/jax_pcache")
        jax.config.update("jax_persistent_cache_min_compile_time_secs", 0)
    except Exception:
        pass
    from concourse import mybir
    from concourse.bass2jax import (install_neuronx_cc_hook, _bass_exec_p,
                                    partition_id_tensor)
    from jax.sharding import Mesh, PartitionSpec, NamedSharding
    from jax.experimental.shard_map import shard_map

    nc = build_program(T)
    install_neuronx_cc_hook()

    partition_name = nc.partition_id_tensor.name if nc.partition_id_tensor else None
    in_names, out_names, out_avals = [], [], []
    for alloc in nc.m.functions[0].allocations:
        if not isinstance(alloc, mybir.MemoryLocationSet):
            continue
        name = alloc.memorylocations[0].name
        if alloc.kind == "ExternalInput":
            if name != partition_name:
                in_names.append(name)
        elif alloc.kind == "ExternalOutput":
            out_names.append(name)
            out_avals.append(jax.core.ShapedArray(
                tuple(alloc.tensor_shape), mybir.dt.np(alloc.dtype)))
    n_params = len(in_names)
    n_outs = len(out_avals)
    all_in = list(in_names) + list(out_names)
    if partition_name:
        all_in.append(partition_name)
    donate = tuple(range(n_params, n_params + n_outs))

    def _body(*args):
        operands = list(args)
        if partition_name:
            operands.append(partition_id_tensor())
        return tuple(_bass_exec_p.bind(
            *operands, out_avals=tuple(out_avals), in_names=tuple(all_in),
            out_names=tuple(out_names), lowering_input_output_aliases=(),
            sim_require_finite=True, sim_require_nnan=True, nc=nc))

    devices = jax.devices()[:N_CORES]
    mesh = Mesh(np.asarray(devices), ("core",))
    sharded = jax.jit(
        shard_map(_body, mesh=mesh,
                  in_specs=(PartitionSpec("core"),) * (n_params + n_outs),
                  out_specs=(PartitionSpec("core"),) * n_outs,
                  check_rep=False),
        donate_argnums=donate, keep_unused=True)
    shardspec = NamedSharding(mesh, PartitionSpec("core"))

    _CACHE[T] = (jax, sharded, shardspec, in_names, out_avals, out_names)
    return _CACHE[T]


def _pack_chunk(sim4, j, qsel, T, R, colm_row):
    """Pack slots 4j..4j+4. sim4: [Q, P, 32, 32] f32. For slot l, ship only
    the first T[l] rows of each 32x32 block for query qsel[core, l]; rows
    >= R[q] and cols >= s_len[p] are zeroed (their marginal weight is the
    1e-5 regularizer) so the transport compressor shrinks them. Returns the
    global [1024, chunk_bytes] u8 array, partition-major per core."""
    nb = sum(int(T[l]) * 24 for l in range(QCH * j, QCH * (j + 1)))
    outb = np.empty((N_CORES, 128, nb), np.uint8)
    off = 0
    for l in range(QCH * j, QCH * (j + 1)):
        tl = int(T[l])
        qs = qsel[:, l]
        blk = sim4[qs, :, :tl, :]                  # [8, 128, tl, 32]
        qv = np.empty((N_CORES, 128, tl, 32), np.uint8)
        # colm_row is QLEVELS-or-0 per (proto, j): quantize and column-mask
        # in one fused pass (sim*0 -> 0 == masked, exact)
        np.multiply(blk, colm_row[None, :, None, :], out=qv, casting="unsafe")
        rm = np.where(np.arange(tl)[None, :] < R[qs][:, None], 63, 0)
        np.bitwise_and(qv, rm.astype(np.uint8)[:, None, :, None], out=qv)
        g = qv.reshape(N_CORES, 128, tl * 8, 4)
        kpb = outb[:, :, off:off + tl * 24].reshape(N_CORES, 128, tl * 8, 3)
        t = np.left_shift(g[..., 1], 6)
        np.bitwise_or(g[..., 0], t, out=kpb[..., 0])
        np.right_shift(g[..., 1], 2, out=t)
        t2 = np.left_shift(g[..., 2], 4)
        np.bitwise_or(t, t2, out=kpb[..., 1])
        np.right_shift(g[..., 2], 4, out=t)
        np.left_shift(g[..., 3], 2, out=t2)
        np.bitwise_or(t, t2, out=kpb[..., 2])
        off += tl * 24
    return outb.reshape(N_CORES * 128, nb)


def kernel(similarity_map, im_set, s_seq, im_len, s_len):
    sim = np.asarray(similarity_map, dtype=np.float32)
    im_l = np.asarray(im_len)
    s_l = np.asarray(s_len)
    a = _marginals(im_l, N1)                        # [128, 32]
    b = _marginals(s_l, N2)                         # [128, 32]

    # Effective row/col counts (length 0 = uniform marginals = keep all),
    # sorted query deal: rank r -> (core r % 8, slot r // 8), so each slot's
    # row budget T[l] = max over its 8 cores hugs the mean (~4% padding).
    R = np.where(im_l == 0, 32, im_l).astype(np.int64)
    C = np.where(s_l == 0, 32, s_l).astype(np.int64)
    order = np.argsort(-R, kind="stable")
    qsel = order.reshape(QL, N_CORES).T             # [core, slot] -> query
    T = R[qsel].max(axis=0)                         # [16] row budgets
    jax, sharded, shardspec, in_names, out_avals, out_names = _get_exec(
        tuple(int(t) for t in T))

    colm_row = np.where(np.arange(32)[None, :] < C[:, None],
                        np.float32(QLEVELS), np.float32(0.0)).astype(np.float32)
    sim4 = sim.reshape(Q, P, N1, N2)

    # pack + transfer pipeline: device_put is async, so chunk j+1 packs on
    # the host while chunk j streams over the tunnel. Small tensors and the
    # smallest chunk go first to minimize exposed pack time.
    zeros = [jax.device_put(
        np.zeros((N_CORES * z.shape[0],) + tuple(z.shape[1:]), z.dtype),
        shardspec) for z in out_avals]
    marg = np.empty((N_CORES, 128 + QL, N2), np.float32)
    marg[:, :128] = b
    marg[:, 128:] = a[qsel]
    resident = {"marg": jax.device_put(marg.reshape(-1, N2), shardspec)}
    for j in range(NCHUNK - 1, -1, -1):
        resident[f"kp{j}"] = jax.device_put(
            _pack_chunk(sim4, j, qsel, T, R, colm_row), shardspec)

    outs = sharded(*[resident[n] for n in in_names], *zeros)

    out = np.asarray(outs[0])                       # [1024, QL] = [8*128, 16]
    blk = out.reshape(N_CORES, 128, QL).transpose(0, 2, 1).reshape(Q, P)
    logits = np.empty((Q, P), np.float32)
    logits[qsel.reshape(-1)] = blk                  # undo the sorted deal
    return logits
